# revision 40
# baseline (speedup 1.0000x reference)
"""BiMamba Trainium2 kernel.

Sharding: 8 cores = (batch 4) x (d-half 2). Every core runs the same SPMD
program: full input projection + depthwise conv + delta/B/C projections
(over all 512 internal channels), then the bidirectional selective scan for
its 256-channel d-half (both directions), gating, and a partial output
projection. The host sums the two partial outputs per batch element.

The d-axis of all weights is permuted per core so that the core's d-half
always occupies channels 0..255 - this keeps the program identical across
cores (pure SPMD, only the input data differs). Inputs are passed
pre-transposed ([d_in, l]) and the depthwise conv kernel as per-tile
diagonal matrices so the conv runs as PSUM-accumulated matmuls.

Scan: h[t] = exp(dA[t]) * h[t-1] + delta*u*B[t] via the DVE
tensor_tensor_scan primitive. Each scan instruction is [128, 4L] packing
(2 n-states) x (2 directions); the decay is zeroed at block boundaries,
which is exact because each recurrence starts from 0. The bwd direction is
computed in reversed coordinates (s = L-1-t) so every scan runs with
forward access patterns (reverse-AP scans are ~1.5x slower); its inputs use
reversed zu/delta and natural-order B/C (matching the reference, which
does not flip B/C for the bwd pass), and the result is un-reversed by a
reversed PSUM read at gating. The per-state readout products are
accumulated into PSUM with identity matmuls on the tensor engine; the
xs*D_param skip term is folded into the same accumulators via diagonal
matmuls. All elementwise work runs on the vector engine (gpsimd degrades
both itself and concurrent DVE work ~2-3x; scalar+DVE coexist cleanly),
and the B/C broadcast DMA bursts are scheduled to land in the scan windows
(scans are latency-bound and tolerate SBUF contention; 2x-mode
tensor_tensors do not).
"""

import sys

for _p in ("/opt/trn_rl_repo",):
    if _p not in sys.path:
        sys.path.insert(0, _p)

from contextlib import ExitStack

import numpy as np
import ml_dtypes

B_SZ, L, D_IN, D_INT = 4, 1024, 256, 512
N_ST, DTR, D_CONV = 16, 16, 4
P = 128
DH = D_INT // 2        # d channels per core (256)
NDT = DH // P          # d-tiles per core in the scan (2)
N_CORES = 8

_cache = {}


class TileCtx:
    """TileContext plus an ExitStack closed before the context exits."""

    def __init__(self, tile_mod, nc):
        self._tc = tile_mod.TileContext(nc)
        self._st = ExitStack()

    def __enter__(self):
        tc = self._tc.__enter__()
        return tc, self._st

    def __exit__(self, *exc):
        self._st.close()
        return self._tc.__exit__(*exc)


def _build_program():
    import concourse.bacc as bacc
    import concourse.tile as tile
    import concourse.mybir as mybir
    from concourse import masks

    dt = mybir.dt
    ST = dt.bfloat16
    f32r = dt.float32r
    Alu = mybir.AluOpType
    AF = mybir.ActivationFunctionType

    nc = bacc.Bacc()

    dd_d = nc.dram_tensor("ddiag", (NDT, P, P), dt.bfloat16, kind="ExternalInput")
    inpT_d = nc.dram_tensor("inpT", (D_IN, L), dt.bfloat16, kind="ExternalInput")
    w_in_d = nc.dram_tensor("w_in", (D_IN, 4 * D_IN), dt.bfloat16, kind="ExternalInput")
    ckd_d = nc.dram_tensor("ckd", (4, D_CONV, P, P), dt.bfloat16, kind="ExternalInput")
    cb_d = nc.dram_tensor("cb", (D_INT, 1), dt.float32, kind="ExternalInput")
    w_x_d = nc.dram_tensor("w_x", (D_INT, DTR + 4 * N_ST), dt.bfloat16, kind="ExternalInput")
    w_dt_d = nc.dram_tensor("w_dt", (DTR, DH), dt.bfloat16, kind="ExternalInput")
    bdt_d = nc.dram_tensor("bdt", (DH, 1), dt.float32, kind="ExternalInput")
    a_d = nc.dram_tensor("a", (DH, N_ST), dt.float32, kind="ExternalInput")
    w_out_d = nc.dram_tensor("w_out", (4, P, D_IN), dt.bfloat16, kind="ExternalInput")
    out_d = nc.dram_tensor("out_part", (L, D_IN), dt.float32, kind="ExternalOutput")

    NLC = L // P           # l-chunks (8)
    NKT = D_IN // P        # k-tiles of the input dim (2)
    NX = DTR + 4 * N_ST    # x_dbl rows (80)
    J_X = list(range(4))   # x_and_res column tiles: x part
    J_R = [4, 5]           # res tiles of our (permuted-to-front) d-half
    L4 = 4 * L

    with TileCtx(tile, nc) as (tc, st):
        cpool = st.enter_context(tc.tile_pool(name="consts", bufs=1))
        main = st.enter_context(tc.tile_pool(name="main", bufs=1))
        drp = st.enter_context(tc.tile_pool(name="dr", bufs=1, space="DRAM"))
        scratch = drp.tile([4 * N_ST, L], ST, name="scratch")

        # ---------------- constants / weights ----------------
        ident16 = cpool.tile([P, P], ST, name="ident16")
        masks.make_identity(nc, ident16[:])

        ckd_all = cpool.tile([P, 4 * D_CONV * P], ST, name="ckd_all")
        ckd_sb = [ckd_all[:, t * D_CONV * P:(t + 1) * D_CONV * P] for t in range(4)]
        cb_all = cpool.tile([P, 4], dt.float32, name="cb_all")
        cb_sb = [cb_all[:, t:t + 1] for t in range(4)]
        wx_all = cpool.tile([P, 4 * NX], ST, name="wx_all")
        w_x_sb = [wx_all[:, t * NX:(t + 1) * NX] for t in range(4)]
        w_dt_sb = cpool.tile([DTR, DH], ST, name="w_dt_sb")
        ba_all = cpool.tile([P, 2 * (N_ST + 1)], dt.float32, name="ba_all")
        bdt_sb = [ba_all[:, t:t + 1] for t in range(NDT)]
        a_sb = [ba_all[:, 2 + t * N_ST:2 + (t + 1) * N_ST] for t in range(NDT)]
        wo_all = cpool.tile([P, 4 * D_IN], ST, name="wo_all")
        w_out_sb = [wo_all[:, t * D_IN:(t + 1) * D_IN] for t in range(4)]
        dd_all = cpool.tile([P, NDT * P], ST, name="dd_all")
        ddiag = [dd_all[:, t * P:(t + 1) * P] for t in range(NDT)]

        # persistent activations (core's d-half only)
        sres16 = [main.tile([P, L], ST, name=f"sres{i}", tag=f"sres{i}") for i in range(2)]
        delta = [main.tile([P, L], ST, name=f"delta{t}", tag=f"delta{t}") for t in range(NDT)]
        xs16 = [main.tile([P, L], ST, name=f"xs16{t}", tag=f"xs16{t}") for t in range(NDT)]
        xs16r = [main.tile([P, L], ST, name=f"xs16r{t}", tag=f"xs16r{t}") for t in range(NDT)]
        # [zu|zu|rev(zu)|rev(zu)] masters: the dbu multiply is one contiguous
        # 2D bf16 tensor_tensor per d-tile per round
        zu4 = [main.tile([P, L4], ST, name=f"zu4{t}", tag=f"zu4{t}") for t in range(NDT)]
        rT = [main.tile([P, L], dt.float32, name=f"rT{i}", tag=f"rT{i}") for i in range(2)]
        gated = {}
        for di in range(2):
            for t in range(NDT):
                gated[(di, t)] = main.tile([P, L], ST, name=f"gated{di}{t}", tag=f"g8{di}{t}")

        # ============ phase 1: projections, conv, delta ============
        with (
            tc.tile_pool(name="pre", bufs=1) as pre,
            tc.tile_pool(name="tmp", bufs=2) as tmp,
            tc.tile_pool(name="psB", bufs=2, space="PSUM") as psB,
            tc.tile_pool(name="psC", bufs=2, space="PSUM") as psC,
        ):
            # loads that gate the first matmuls go out first
            inpT16 = [pre.tile([P, L], ST, name=f"inpT16{k}", tag=f"inpT16{k}") for k in range(NKT)]
            w_in16 = [pre.tile([P, 4 * D_IN], ST, name=f"wi16{k}", tag=f"wi16{k}") for k in range(NKT)]
            for k in range(NKT):
                nc.sync.dma_start(inpT16[k][:], inpT_d[k * P:(k + 1) * P, :])
                nc.scalar.dma_start(w_in16[k][:], w_in_d[k * P:(k + 1) * P, :])
            nc.sync.dma_start(
                ckd_all[:].rearrange("p (t w q) -> p t w q", t=4, w=D_CONV),
                ckd_d[:, :, :, :].transpose([2, 0, 1, 3]))
            nc.scalar.dma_start(cb_all[:].rearrange("p (t o) -> p t o", t=4),
                                cb_d[:, :].rearrange("(t p) o -> p t o", t=4))
            nc.scalar.dma_start(wx_all[:].rearrange("p (t x) -> p t x", t=4),
                                w_x_d[:, :].rearrange("(t p) x -> p t x", t=4))
            nc.scalar.dma_start(w_dt_sb[:], w_dt_d[:])
            nc.scalar.dma_start(ba_all[:, 0:2].rearrange("p (t o) -> p t o", t=NDT),
                                bdt_d[:, :].rearrange("(t p) o -> p t o", t=NDT))
            nc.scalar.dma_start(ba_all[:, 2:].rearrange("p (t n) -> p t n", t=NDT),
                                a_d[:, :].rearrange("(t p) n -> p t n", t=NDT))
            nc.sync.dma_start(wo_all[:].rearrange("p (t d) -> p t d", t=4),
                              w_out_d[:, :, :].transpose([1, 0, 2]))
            nc.sync.dma_start(dd_all[:].rearrange("p (t q) -> p t q", t=NDT),
                              dd_d[:, :, :].transpose([1, 0, 2]))

            xpad = [pre.tile([P, L + 3], ST, name=f"xpad{t}", tag=f"xpad{t}") for t in range(4)]
            for t in range(4):
                nc.vector.memset(xpad[t][:, 0:1], 0.0)
                nc.vector.memset(xpad[t][:, L + 1:L + 3], 0.0)
            # bf16 x tiles: t<2 are this core's d-half (reused as xs16)
            xs_all = xs16 + [pre.tile([P, L], ST, name=f"xsh{t}", tag=f"xsh{t}")
                             for t in range(2)]

            # x_and_res^T = W_in^T @ inputs^T (bf16), conv interleaved per
            # d-tile so the x path reaches delta as early as possible
            def inproj(j, dest):
                mm = psB.tile([P, L], dt.float32, name="mm", tag="mm")
                for lh in range(2):
                    for k in range(NKT):
                        nc.tensor.matmul(
                            mm[:, lh * 512:(lh + 1) * 512],
                            w_in16[k][:, j * P:(j + 1) * P],
                            inpT16[k][:, lh * 512:(lh + 1) * 512],
                            start=(k == 0), stop=(k == NKT - 1))
                nc.vector.tensor_copy(dest, mm[:])

            for t in range(4):
                inproj(t, xpad[t][:, 1:1 + L])
                cm = psC.tile([P, L], dt.float32, name="cm", tag="cm")
                for lh in range(2):
                    for w in range(D_CONV):
                        nc.tensor.matmul(
                            cm[:, lh * 512:(lh + 1) * 512],
                            ckd_sb[t][:, w * P:(w + 1) * P],
                            xpad[t][:, w + lh * 512:w + lh * 512 + 512],
                            start=(w == 0), stop=(w == D_CONV - 1))
                nc.scalar.activation(xs_all[t][:], cm[:], AF.Silu,
                                     bias=cb_sb[t][:], scale=1.0)

            # x_dbl^T = W_x^T @ xs  (bf16)
            xdb = pre.tile([NX, L], ST, name="xdb")
            mmx = psB.tile([NX, L], dt.float32, name="mmx", tag="mm")
            for lh in range(2):
                for t in range(4):
                    nc.tensor.matmul(mmx[:, lh * 512:(lh + 1) * 512], w_x_sb[t][:],
                                     xs_all[t][:, lh * 512:(lh + 1) * 512],
                                     start=(t == 0), stop=(t == 3))
            nc.vector.tensor_copy(xdb[:], mmx[:])

            # delta = softplus(x_dbl[:, :16] @ W_dt + b_dt)
            for t in range(NDT):
                mm = psB.tile([P, L], dt.float32, name="mmd", tag="mm")
                for lh in range(2):
                    nc.tensor.matmul(mm[:, lh * 512:(lh + 1) * 512],
                                     w_dt_sb[:, t * P:(t + 1) * P],
                                     xdb[0:DTR, lh * 512:(lh + 1) * 512],
                                     start=True, stop=True)
                # softplus(pre + b_dt) = ln(1 + exp(pre + b_dt))
                et = tmp.tile([P, L], dt.float32, name="et", tag="et")
                nc.scalar.activation(et[:], mm[:], AF.Exp, bias=bdt_sb[t][:], scale=1.0)
                nc.scalar.activation(delta[t][:], et[:], AF.Ln, bias=1.0, scale=1.0)

            # zu4 = delta*xs written quarter-wise
            for t in range(NDT):
                for q in range(4):
                    dsrc = delta[t][:] if q < 2 else delta[t][:, ::-1]
                    xsrc = xs16[t][:] if q < 2 else xs16[t][:, ::-1]
                    nc.vector.tensor_tensor(zu4[t][:, q * L:(q + 1) * L],
                                            dsrc, xsrc, Alu.mult)
                nc.scalar.copy(xs16r[t][:], xs16[t][:, ::-1])

            # stage B/C rows to DRAM (all natural time order) for broadcast
            nc.scalar.dma_start(scratch[0:4 * N_ST, :],
                                xdb[DTR:DTR + 4 * N_ST, :])

            # res projection (silu deferred into phase 2; only gating needs it)
            for i in range(2):
                inproj(4 + i, rT[i][:])

        # ============ phase 2: bidirectional selective scan ============
        # Round structure: dbu x2 -> scan x2 -> g x2 (+readout matmuls), so
        # the next round's broadcast DMAs land in the scan window. Each
        # [128, 4L] tile packs quarters [fwd n0 | fwd n1 | bwd n0 | bwd n1],
        # with the bwd quarters in reversed coordinates.
        with (
            tc.tile_pool(name="ypsum", bufs=1, space="PSUM") as yps,
            tc.tile_pool(name="dbup", bufs=1) as dbup,
            tc.tile_pool(name="hp", bufs=1) as hp,
            tc.tile_pool(name="gp", bufs=1) as gp,
            tc.tile_pool(name="epool", bufs=2) as ep,
            tc.tile_pool(name="bcB", bufs=2) as bcB,
            tc.tile_pool(name="bcC", bufs=2) as bcC,
        ):
            ypt = {}
            for di in range(2):
                for t in range(NDT):
                    ypt[(di, t)] = yps.tile([P, L], dt.float32,
                                            name=f"y{di}{t}", tag=f"y{di}{t}")

            # seed the readout accumulators with the xs * D_param skip term
            # (diagonal stationary; bwd uses reversed xs to match s-coords)
            for di in range(2):
                for t in range(NDT):
                    xsrc = xs16[t] if di == 0 else xs16r[t]
                    for lh in range(2):
                        nc.tensor.matmul(ypt[(di, t)][:, lh * 512:(lh + 1) * 512],
                                         ddiag[t][:],
                                         xsrc[:, lh * 512:(lh + 1) * 512],
                                         start=True, stop=False)

            NG = N_ST // 2   # n-pair groups

            def issue_bcast(g8):
                B4 = bcB.tile([P, L4], ST, name="B4", tag="B4")
                C4 = bcC.tile([P, L4], ST, name="C4", tag="C4")
                for di in range(2):
                    bsrc = scratch[di * N_ST + 2 * g8:di * N_ST + 2 * g8 + 2, :]
                    nc.sync.dma_start(
                        B4[:, di * 2 * L:(di + 1) * 2 * L]
                        .rearrange("p (g l) -> p g l", g=2),
                        bsrc.unsqueeze(0).broadcast_to([P, 2, L]))
                for di in range(2):
                    csrc = scratch[(2 + di) * N_ST + 2 * g8:
                                   (2 + di) * N_ST + 2 * g8 + 2, :]
                    nc.sync.dma_start(
                        C4[:, di * 2 * L:(di + 1) * 2 * L]
                        .rearrange("p (g l) -> p g l", g=2),
                        csrc.unsqueeze(0).broadcast_to([P, 2, L]))
                return B4, C4

            bcast_q = [issue_bcast(0)]
            for g8 in range(NG):
                B4, C4 = bcast_q.pop(0)
                if g8 + 1 < NG:
                    bcast_q.append(issue_bcast(g8 + 1))

                # decay tiles (scalar): quarters 0,1 natural; 2,3 reversed.
                # Round 0's were produced at the end of phase 1.
                Es = {}
                for t in range(NDT):
                    E = ep.tile([P, L4], ST, name=f"E{t}", tag=f"E{t}")
                    for q in range(4):
                        dsrc = delta[t][:] if q < 2 else delta[t][:, ::-1]
                        acol = a_sb[t][:, 2 * g8 + (q % 2):2 * g8 + (q % 2) + 1]
                        nc.scalar.activation(E[:, q * L:(q + 1) * L], dsrc,
                                             AF.Exp, bias=0.0, scale=acol)
                        if q:
                            # reset the recurrence at the block boundary
                            nc.scalar.mul(E[:, q * L:q * L + 1],
                                          E[:, q * L:q * L + 1], 0.0)
                    Es[t] = E
                if g8 == 0:
                    # silu(res) directly on the scalar engine, tucked into the
                    # first round's scan window
                    for i in range(2):
                        nc.scalar.activation(sres16[i][:], rT[i][:], AF.Silu)

                def emit_dbu(t):
                    db = dbup.tile([P, L4], ST, name=f"dbu{t}", tag=f"dbu{t}")
                    nc.vector.tensor_tensor(db[:], zu4[t][:], B4[:], Alu.mult)
                    return db

                def emit_scan(t, db):
                    h = hp.tile([P, L4], ST, name=f"h{t}", tag=f"h{t}")
                    nc.vector.tensor_tensor_scan(h[:], Es[t][:], db[:],
                                                 0.0, Alu.mult, Alu.add)
                    return h

                def emit_g(t, h):
                    # g in its own buffer, so the next round's dbu does not
                    # wait on the readout matmuls; its readouts overlap the
                    # other tile's scan
                    g = gp.tile([P, L4], ST, name=f"g{t}", tag=f"g{t}")
                    nc.vector.tensor_tensor(g[:], h[:], C4[:], Alu.mult)
                    for di in range(2):
                        for nb in range(2):
                            q = 2 * di + nb
                            for lh in range(2):
                                nc.tensor.matmul(
                                    ypt[(di, t)][:, lh * 512:(lh + 1) * 512],
                                    ident16[:],
                                    g[:, q * L + lh * 512:q * L + (lh + 1) * 512],
                                    start=False,
                                    stop=(g8 == NG - 1 and nb == 1))

                db0 = emit_dbu(0)
                h0 = emit_scan(0, db0)
                db1 = emit_dbu(1)
                emit_g(0, h0)
                h1 = emit_scan(1, db1)
                emit_g(1, h1)
                if g8 == NG - 1:
                    # gating: gated = (xs*D + y_scan) * silu(res); xs*D is
                    # already in PSUM, bwd is read fully reversed to undo the
                    # s-coordinates. Emitted per d-tile right after its final
                    # readout matmuls so it overlaps the remaining work.
                    # split by column halves, first halves first, so the
                    # output projection's low chunks start while the second
                    # halves are still gating
                    for lh in range(2):
                        for t in range(NDT):
                            for di in range(2):
                                ysrc = (ypt[(di, t)][:] if di == 0
                                        else ypt[(di, t)][:, ::-1])
                                nc.vector.tensor_tensor(
                                    gated[(di, t)][:, lh * 512:(lh + 1) * 512],
                                    ysrc[:, lh * 512:(lh + 1) * 512],
                                    sres16[t][:, lh * 512:(lh + 1) * 512],
                                    Alu.mult)

        # ============ phase 3: output projection (bf16) ============
        with (
            tc.tile_pool(name="ops", bufs=3, space="PSUM") as ops,
            tc.tile_pool(name="osb", bufs=3) as osb,
        ):
            for c in range(NLC):
                om = ops.tile([P, D_IN], dt.float32, name="om", tag="om")
                idx = 0
                for di in range(2):
                    for t in range(NDT):
                        nc.tensor.matmul(om[:], gated[(di, t)][:, c * P:(c + 1) * P],
                                         w_out_sb[di * NDT + t][:],
                                         start=(idx == 0), stop=(idx == 3))
                        idx += 1
                ot = osb.tile([P, D_IN], dt.float32, name="ot", tag="ot")
                nc.vector.tensor_copy(ot[:], om[:])
                nc.sync.dma_start(out_d[c * P:(c + 1) * P, :], ot[:])

    nc.finalize()
    return nc


def _shard_inputs(inputs, W_in, conv_k, conv_b, W_x, W_dt, b_dt, A_log, D_param, W_out):
    f32 = np.float32
    inputs = np.asarray(inputs, f32)
    W_in = np.asarray(W_in, f32)
    ck = np.asarray(conv_k, f32).reshape(D_CONV, D_INT)
    cb = np.asarray(conv_b, f32)
    W_x = np.asarray(W_x, f32)
    W_dt = np.asarray(W_dt, f32)
    b_dt = np.asarray(b_dt, f32)
    A = -np.exp(np.asarray(A_log, f32))
    D_param = np.asarray(D_param, f32)
    W_out = np.asarray(W_out, f32)

    in_maps = []
    for core in range(N_CORES):
        b, dh = divmod(core, 2)
        perm = np.concatenate([np.arange(dh * DH, (dh + 1) * DH),
                               np.arange((1 - dh) * DH, (2 - dh) * DH)])
        half = perm[:DH]
        w_in_p = np.concatenate([W_in[:, :D_INT][:, perm], W_in[:, D_INT:][:, perm]],
                                axis=1)
        ckp = ck[:, perm]                      # [4, 512]
        ckd = np.zeros((4, D_CONV, P, P), f32)
        for t in range(4):
            for w in range(D_CONV):
                np.fill_diagonal(ckd[t, w], ckp[w, t * P:(t + 1) * P])
        w_out4 = np.stack([
            W_out[half[0:P]], W_out[half[P:2 * P]],
            W_out[D_INT + half[0:P]], W_out[D_INT + half[P:2 * P]],
        ])
        dd = np.zeros((NDT, P, P), f32)
        for t in range(NDT):
            np.fill_diagonal(dd[t], D_param[half[t * P:(t + 1) * P]])
        bf16 = ml_dtypes.bfloat16
        in_maps.append({
            "ddiag": dd.astype(bf16),
            "inpT": np.ascontiguousarray(inputs[b].T).astype(bf16),
            "w_in": np.ascontiguousarray(w_in_p).astype(bf16),
            "ckd": ckd.astype(bf16),
            "cb": np.ascontiguousarray(cb[perm][:, None]),
            "w_x": np.ascontiguousarray(W_x[perm]).astype(bf16),
            "w_dt": np.ascontiguousarray(W_dt[:, half]).astype(bf16),
            "bdt": np.ascontiguousarray(b_dt[half][:, None]),
            "a": np.ascontiguousarray(A[half]),
            "w_out": np.ascontiguousarray(w_out4).astype(bf16),
        })
    return in_maps


LAST_EXEC_NS = None


def kernel(**inputs):
    global LAST_EXEC_NS
    import os
    from concourse.bass_utils import run_bass_kernel_spmd

    if "nc" not in _cache:
        _cache["nc"] = _build_program()
    nc = _cache["nc"]
    in_maps = _shard_inputs(**inputs)
    trace = bool(int(os.environ.get("BIMAMBA_TRACE", "0")))
    res = run_bass_kernel_spmd(nc, in_maps, core_ids=list(range(N_CORES)), trace=trace)
    _cache["last_res"] = res
    LAST_EXEC_NS = res.exec_time_ns
    out = np.zeros((B_SZ, L, D_IN), np.float32)
    for b in range(B_SZ):
        out[b] = res.results[2 * b]["out_part"] + res.results[2 * b + 1]["out_part"]
    return out


# revision 41
# speedup vs baseline: 1.0034x; 1.0034x over previous
"""BiMamba Trainium2 kernel.

Sharding: 8 cores = (batch 4) x (d-half 2). Every core runs the same SPMD
program: full input projection + depthwise conv + delta/B/C projections
(over all 512 internal channels), then the bidirectional selective scan for
its 256-channel d-half (both directions), gating, and a partial output
projection. The host sums the two partial outputs per batch element.

The d-axis of all weights is permuted per core so that the core's d-half
always occupies channels 0..255 - this keeps the program identical across
cores (pure SPMD, only the input data differs). Inputs are passed
pre-transposed ([d_in, l]) and the depthwise conv kernel as per-tile
diagonal matrices so the conv runs as PSUM-accumulated matmuls.

Scan: h[t] = exp(dA[t]) * h[t-1] + delta*u*B[t] via the DVE
tensor_tensor_scan primitive. Each scan instruction is [128, 4L] packing
(2 n-states) x (2 directions); the decay is zeroed at block boundaries,
which is exact because each recurrence starts from 0. The bwd direction is
computed in reversed coordinates (s = L-1-t) so every scan runs with
forward access patterns (reverse-AP scans are ~1.5x slower); its inputs use
reversed zu/delta and natural-order B/C (matching the reference, which
does not flip B/C for the bwd pass), and the result is un-reversed by a
reversed PSUM read at gating. The per-state readout products are
accumulated into PSUM with identity matmuls on the tensor engine; the
xs*D_param skip term is folded into the same accumulators via diagonal
matmuls. All elementwise work runs on the vector engine (gpsimd degrades
both itself and concurrent DVE work ~2-3x; scalar+DVE coexist cleanly),
and the B/C broadcast DMA bursts are scheduled to land in the scan windows
(scans are latency-bound and tolerate SBUF contention; 2x-mode
tensor_tensors do not).
"""

import sys

for _p in ("/opt/trn_rl_repo",):
    if _p not in sys.path:
        sys.path.insert(0, _p)

from contextlib import ExitStack

import numpy as np
import ml_dtypes

B_SZ, L, D_IN, D_INT = 4, 1024, 256, 512
N_ST, DTR, D_CONV = 16, 16, 4
P = 128
DH = D_INT // 2        # d channels per core (256)
NDT = DH // P          # d-tiles per core in the scan (2)
N_CORES = 8

_cache = {}


class TileCtx:
    """TileContext plus an ExitStack closed before the context exits."""

    def __init__(self, tile_mod, nc):
        self._tc = tile_mod.TileContext(nc)
        self._st = ExitStack()

    def __enter__(self):
        tc = self._tc.__enter__()
        return tc, self._st

    def __exit__(self, *exc):
        self._st.close()
        return self._tc.__exit__(*exc)


def _build_program():
    import concourse.bacc as bacc
    import concourse.tile as tile
    import concourse.mybir as mybir
    from concourse import masks

    dt = mybir.dt
    ST = dt.bfloat16
    f32r = dt.float32r
    Alu = mybir.AluOpType
    AF = mybir.ActivationFunctionType

    nc = bacc.Bacc()

    dd_d = nc.dram_tensor("ddiag", (NDT, P, P), dt.bfloat16, kind="ExternalInput")
    inpT_d = nc.dram_tensor("inpT", (D_IN, L), dt.bfloat16, kind="ExternalInput")
    w_in_d = nc.dram_tensor("w_in", (D_IN, 4 * D_IN), dt.bfloat16, kind="ExternalInput")
    ckd_d = nc.dram_tensor("ckd", (4, D_CONV, P, P), dt.bfloat16, kind="ExternalInput")
    cb_d = nc.dram_tensor("cb", (D_INT, 1), dt.float32, kind="ExternalInput")
    w_x_d = nc.dram_tensor("w_x", (D_INT, DTR + 4 * N_ST), dt.bfloat16, kind="ExternalInput")
    w_dt_d = nc.dram_tensor("w_dt", (DTR, DH), dt.bfloat16, kind="ExternalInput")
    bdt_d = nc.dram_tensor("bdt", (DH, 1), dt.float32, kind="ExternalInput")
    a_d = nc.dram_tensor("a", (DH, N_ST), dt.float32, kind="ExternalInput")
    w_out_d = nc.dram_tensor("w_out", (4, P, D_IN), dt.bfloat16, kind="ExternalInput")
    out_d = nc.dram_tensor("out_part", (L, D_IN), dt.float32, kind="ExternalOutput")

    NLC = L // P           # l-chunks (8)
    NKT = D_IN // P        # k-tiles of the input dim (2)
    NX = DTR + 4 * N_ST    # x_dbl rows (80)
    J_X = list(range(4))   # x_and_res column tiles: x part
    J_R = [4, 5]           # res tiles of our (permuted-to-front) d-half
    L4 = 4 * L

    with TileCtx(tile, nc) as (tc, st):
        cpool = st.enter_context(tc.tile_pool(name="consts", bufs=1))
        main = st.enter_context(tc.tile_pool(name="main", bufs=1))
        drp = st.enter_context(tc.tile_pool(name="dr", bufs=1, space="DRAM"))
        scratch = drp.tile([4 * N_ST, L], ST, name="scratch")

        # ---------------- constants / weights ----------------
        ident16 = cpool.tile([P, P], ST, name="ident16")
        masks.make_identity(nc, ident16[:])

        ckd_all = cpool.tile([P, 4 * D_CONV * P], ST, name="ckd_all")
        ckd_sb = [ckd_all[:, t * D_CONV * P:(t + 1) * D_CONV * P] for t in range(4)]
        cb_all = cpool.tile([P, 4], dt.float32, name="cb_all")
        cb_sb = [cb_all[:, t:t + 1] for t in range(4)]
        wx_all = cpool.tile([P, 4 * NX], ST, name="wx_all")
        w_x_sb = [wx_all[:, t * NX:(t + 1) * NX] for t in range(4)]
        w_dt_sb = cpool.tile([DTR, DH], ST, name="w_dt_sb")
        ba_all = cpool.tile([P, 2 * (N_ST + 1)], dt.float32, name="ba_all")
        bdt_sb = [ba_all[:, t:t + 1] for t in range(NDT)]
        a_sb = [ba_all[:, 2 + t * N_ST:2 + (t + 1) * N_ST] for t in range(NDT)]
        wo_all = cpool.tile([P, 4 * D_IN], ST, name="wo_all")
        w_out_sb = [wo_all[:, t * D_IN:(t + 1) * D_IN] for t in range(4)]
        dd_all = cpool.tile([P, NDT * P], ST, name="dd_all")
        ddiag = [dd_all[:, t * P:(t + 1) * P] for t in range(NDT)]

        # persistent activations (core's d-half only)
        sres16 = [main.tile([P, L], ST, name=f"sres{i}", tag=f"sres{i}") for i in range(2)]
        delta = [main.tile([P, L], ST, name=f"delta{t}", tag=f"delta{t}") for t in range(NDT)]
        xs16 = [main.tile([P, L], ST, name=f"xs16{t}", tag=f"xs16{t}") for t in range(NDT)]
        xs16r = [main.tile([P, L], ST, name=f"xs16r{t}", tag=f"xs16r{t}") for t in range(NDT)]
        # [zu|zu|rev(zu)|rev(zu)] masters: the dbu multiply is one contiguous
        # 2D bf16 tensor_tensor per d-tile per round
        zu4 = [main.tile([P, L4], ST, name=f"zu4{t}", tag=f"zu4{t}") for t in range(NDT)]
        rT = [main.tile([P, L], dt.float32, name=f"rT{i}", tag=f"rT{i}") for i in range(2)]
        gated = {}
        for di in range(2):
            for t in range(NDT):
                gated[(di, t)] = main.tile([P, L], ST, name=f"gated{di}{t}", tag=f"g8{di}{t}")

        # ============ phase 1: projections, conv, delta ============
        with (
            tc.tile_pool(name="pre", bufs=1) as pre,
            tc.tile_pool(name="tmp", bufs=2) as tmp,
            tc.tile_pool(name="psB", bufs=2, space="PSUM") as psB,
            tc.tile_pool(name="psC", bufs=2, space="PSUM") as psC,
        ):
            # loads that gate the first matmuls go out first
            inpT16 = [pre.tile([P, L], ST, name=f"inpT16{k}", tag=f"inpT16{k}") for k in range(NKT)]
            w_in16 = [pre.tile([P, 4 * D_IN], ST, name=f"wi16{k}", tag=f"wi16{k}") for k in range(NKT)]
            for k in range(NKT):
                nc.sync.dma_start(inpT16[k][:], inpT_d[k * P:(k + 1) * P, :])
                nc.scalar.dma_start(w_in16[k][:], w_in_d[k * P:(k + 1) * P, :])
            nc.sync.dma_start(
                ckd_all[:].rearrange("p (t w q) -> p t w q", t=4, w=D_CONV),
                ckd_d[:, :, :, :].transpose([2, 0, 1, 3]))
            nc.scalar.dma_start(cb_all[:].rearrange("p (t o) -> p t o", t=4),
                                cb_d[:, :].rearrange("(t p) o -> p t o", t=4))
            nc.scalar.dma_start(wx_all[:].rearrange("p (t x) -> p t x", t=4),
                                w_x_d[:, :].rearrange("(t p) x -> p t x", t=4))
            nc.scalar.dma_start(w_dt_sb[:], w_dt_d[:])
            nc.scalar.dma_start(ba_all[:, 0:2].rearrange("p (t o) -> p t o", t=NDT),
                                bdt_d[:, :].rearrange("(t p) o -> p t o", t=NDT))
            nc.scalar.dma_start(ba_all[:, 2:].rearrange("p (t n) -> p t n", t=NDT),
                                a_d[:, :].rearrange("(t p) n -> p t n", t=NDT))
            nc.sync.dma_start(wo_all[:].rearrange("p (t d) -> p t d", t=4),
                              w_out_d[:, :, :].transpose([1, 0, 2]))
            nc.sync.dma_start(dd_all[:].rearrange("p (t q) -> p t q", t=NDT),
                              dd_d[:, :, :].transpose([1, 0, 2]))

            xpad = [pre.tile([P, L + 3], ST, name=f"xpad{t}", tag=f"xpad{t}") for t in range(4)]
            for t in range(4):
                nc.vector.memset(xpad[t][:, 0:1], 0.0)
                nc.vector.memset(xpad[t][:, L + 1:L + 3], 0.0)
            # bf16 x tiles: t<2 are this core's d-half (reused as xs16)
            xs_all = xs16 + [pre.tile([P, L], ST, name=f"xsh{t}", tag=f"xsh{t}")
                             for t in range(2)]

            # x_and_res^T = W_in^T @ inputs^T (bf16), conv interleaved per
            # d-tile so the x path reaches delta as early as possible
            def inproj(j, dest):
                mm = psB.tile([P, L], dt.float32, name="mm", tag="mm")
                for lh in range(2):
                    for k in range(NKT):
                        nc.tensor.matmul(
                            mm[:, lh * 512:(lh + 1) * 512],
                            w_in16[k][:, j * P:(j + 1) * P],
                            inpT16[k][:, lh * 512:(lh + 1) * 512],
                            start=(k == 0), stop=(k == NKT - 1))
                nc.vector.tensor_copy(dest, mm[:])

            for t in range(4):
                inproj(t, xpad[t][:, 1:1 + L])
                cm = psC.tile([P, L], dt.float32, name="cm", tag="cm")
                for lh in range(2):
                    for w in range(D_CONV):
                        nc.tensor.matmul(
                            cm[:, lh * 512:(lh + 1) * 512],
                            ckd_sb[t][:, w * P:(w + 1) * P],
                            xpad[t][:, w + lh * 512:w + lh * 512 + 512],
                            start=(w == 0), stop=(w == D_CONV - 1))
                nc.scalar.activation(xs_all[t][:], cm[:], AF.Silu,
                                     bias=cb_sb[t][:], scale=1.0)

            # x_dbl^T = W_x^T @ xs  (bf16)
            xdb = pre.tile([NX, L], ST, name="xdb")
            mmx = psB.tile([NX, L], dt.float32, name="mmx", tag="mm")
            for lh in range(2):
                for t in range(4):
                    nc.tensor.matmul(mmx[:, lh * 512:(lh + 1) * 512], w_x_sb[t][:],
                                     xs_all[t][:, lh * 512:(lh + 1) * 512],
                                     start=(t == 0), stop=(t == 3))
            nc.vector.tensor_copy(xdb[:], mmx[:])

            # delta = softplus(x_dbl[:, :16] @ W_dt + b_dt)
            for t in range(NDT):
                mm = psB.tile([P, L], dt.float32, name="mmd", tag="mm")
                for lh in range(2):
                    nc.tensor.matmul(mm[:, lh * 512:(lh + 1) * 512],
                                     w_dt_sb[:, t * P:(t + 1) * P],
                                     xdb[0:DTR, lh * 512:(lh + 1) * 512],
                                     start=True, stop=True)
                # softplus(pre + b_dt) = ln(1 + exp(pre + b_dt))
                et = tmp.tile([P, L], dt.float32, name="et", tag="et")
                nc.scalar.activation(et[:], mm[:], AF.Exp, bias=bdt_sb[t][:], scale=1.0)
                nc.scalar.activation(delta[t][:], et[:], AF.Ln, bias=1.0, scale=1.0)

            # zu4 = delta*xs written quarter-wise
            for t in range(NDT):
                for q in range(4):
                    dsrc = delta[t][:] if q < 2 else delta[t][:, ::-1]
                    xsrc = xs16[t][:] if q < 2 else xs16[t][:, ::-1]
                    nc.vector.tensor_tensor(zu4[t][:, q * L:(q + 1) * L],
                                            dsrc, xsrc, Alu.mult)
                nc.scalar.copy(xs16r[t][:], xs16[t][:, ::-1])

            # stage B/C rows to DRAM (all natural time order) for broadcast
            nc.scalar.dma_start(scratch[0:4 * N_ST, :],
                                xdb[DTR:DTR + 4 * N_ST, :])

            # res projection (silu deferred into phase 2; only gating needs it)
            for i in range(2):
                inproj(4 + i, rT[i][:])

        # ============ phase 2: bidirectional selective scan ============
        # Round structure: dbu x2 -> scan x2 -> g x2 (+readout matmuls), so
        # the next round's broadcast DMAs land in the scan window. Each
        # [128, 4L] tile packs quarters [fwd n0 | fwd n1 | bwd n0 | bwd n1],
        # with the bwd quarters in reversed coordinates.
        with (
            tc.tile_pool(name="ypsum", bufs=1, space="PSUM") as yps,
            tc.tile_pool(name="dbup", bufs=1) as dbup,
            tc.tile_pool(name="hp", bufs=1) as hp,
            tc.tile_pool(name="gp", bufs=1) as gp,
            tc.tile_pool(name="epool", bufs=2) as ep,
            tc.tile_pool(name="bcB", bufs=2) as bcB,
            tc.tile_pool(name="bcC", bufs=2) as bcC,
        ):
            ypt = {}
            for di in range(2):
                for t in range(NDT):
                    ypt[(di, t)] = yps.tile([P, L], dt.float32,
                                            name=f"y{di}{t}", tag=f"y{di}{t}")

            # seed the readout accumulators with the xs * D_param skip term
            # (diagonal stationary; bwd uses reversed xs to match s-coords)
            for di in range(2):
                for t in range(NDT):
                    xsrc = xs16[t] if di == 0 else xs16r[t]
                    for lh in range(2):
                        nc.tensor.matmul(ypt[(di, t)][:, lh * 512:(lh + 1) * 512],
                                         ddiag[t][:],
                                         xsrc[:, lh * 512:(lh + 1) * 512],
                                         start=True, stop=False)

            NG = N_ST // 2   # n-pair groups

            def issue_bcast(g8):
                B4 = bcB.tile([P, L4], ST, name="B4", tag="B4")
                C4 = bcC.tile([P, L4], ST, name="C4", tag="C4")
                for di in range(2):
                    bsrc = scratch[di * N_ST + 2 * g8:di * N_ST + 2 * g8 + 2, :]
                    nc.sync.dma_start(
                        B4[:, di * 2 * L:(di + 1) * 2 * L]
                        .rearrange("p (g l) -> p g l", g=2),
                        bsrc.unsqueeze(0).broadcast_to([P, 2, L]))
                for di in range(2):
                    csrc = scratch[(2 + di) * N_ST + 2 * g8:
                                   (2 + di) * N_ST + 2 * g8 + 2, :]
                    nc.sync.dma_start(
                        C4[:, di * 2 * L:(di + 1) * 2 * L]
                        .rearrange("p (g l) -> p g l", g=2),
                        csrc.unsqueeze(0).broadcast_to([P, 2, L]))
                return B4, C4

            bcast_q = [issue_bcast(0)]
            for g8 in range(NG):
                B4, C4 = bcast_q.pop(0)
                if g8 + 1 < NG:
                    bcast_q.append(issue_bcast(g8 + 1))

                # decay tiles (scalar): quarters 0,1 natural; 2,3 reversed.
                # Round 0's were produced at the end of phase 1.
                Es = {}
                for t in range(NDT):
                    E = ep.tile([P, L4], ST, name=f"E{t}", tag=f"E{t}")
                    for q in range(4):
                        dsrc = delta[t][:] if q < 2 else delta[t][:, ::-1]
                        acol = a_sb[t][:, 2 * g8 + (q % 2):2 * g8 + (q % 2) + 1]
                        nc.scalar.activation(E[:, q * L:(q + 1) * L], dsrc,
                                             AF.Exp, bias=0.0, scale=acol)
                        if q:
                            # reset the recurrence at the block boundary
                            nc.scalar.mul(E[:, q * L:q * L + 1],
                                          E[:, q * L:q * L + 1], 0.0)
                    Es[t] = E
                if g8 == 0:
                    # silu(res) directly on the scalar engine, tucked into the
                    # first round's scan window
                    for i in range(2):
                        nc.scalar.activation(sres16[i][:], rT[i][:], AF.Silu)

                def emit_dbu(t):
                    db = dbup.tile([P, L4], ST, name=f"dbu{t}", tag=f"dbu{t}")
                    nc.vector.tensor_tensor(db[:], zu4[t][:], B4[:], Alu.mult)
                    return db

                def emit_scan(t, db):
                    h = hp.tile([P, L4], ST, name=f"h{t}", tag=f"h{t}")
                    nc.vector.tensor_tensor_scan(h[:], Es[t][:], db[:],
                                                 0.0, Alu.mult, Alu.add)
                    return h

                def emit_g(t, h):
                    # g in its own buffer, so the next round's dbu does not
                    # wait on the readout matmuls; its readouts overlap the
                    # other tile's scan
                    g = gp.tile([P, L4], ST, name=f"g{t}", tag=f"g{t}")
                    nc.vector.tensor_tensor(g[:], h[:], C4[:], Alu.mult)
                    for di in range(2):
                        for nb in range(2):
                            q = 2 * di + nb
                            for lh in range(2):
                                nc.tensor.matmul(
                                    ypt[(di, t)][:, lh * 512:(lh + 1) * 512],
                                    ident16[:],
                                    g[:, q * L + lh * 512:q * L + (lh + 1) * 512],
                                    start=False,
                                    stop=(g8 == NG - 1 and nb == 1))

                db0 = emit_dbu(0)
                h0 = emit_scan(0, db0)
                db1 = emit_dbu(1)
                emit_g(0, h0)
                h1 = emit_scan(1, db1)
                emit_g(1, h1)
                if g8 == NG - 1:
                    # gating: gated = (xs*D + y_scan) * silu(res); xs*D is
                    # already in PSUM, bwd is read fully reversed to undo the
                    # s-coordinates. Emitted per d-tile right after its final
                    # readout matmuls so it overlaps the remaining work.
                    for t in range(NDT):
                        for di in range(2):
                            ysrc = (ypt[(di, t)][:] if di == 0
                                    else ypt[(di, t)][:, ::-1])
                            nc.vector.tensor_tensor(gated[(di, t)][:], ysrc,
                                                    sres16[t][:], Alu.mult)

        # ============ phase 3: output projection (bf16) ============
        with (
            tc.tile_pool(name="ops", bufs=3, space="PSUM") as ops,
            tc.tile_pool(name="osb", bufs=3) as osb,
        ):
            for c in range(NLC):
                om = ops.tile([P, D_IN], dt.float32, name="om", tag="om")
                idx = 0
                for di in range(2):
                    for t in range(NDT):
                        nc.tensor.matmul(om[:], gated[(di, t)][:, c * P:(c + 1) * P],
                                         w_out_sb[di * NDT + t][:],
                                         start=(idx == 0), stop=(idx == 3))
                        idx += 1
                ot = osb.tile([P, D_IN], dt.float32, name="ot", tag="ot")
                nc.vector.tensor_copy(ot[:], om[:])
                nc.sync.dma_start(out_d[c * P:(c + 1) * P, :], ot[:])

    nc.finalize()
    return nc


def _shard_inputs(inputs, W_in, conv_k, conv_b, W_x, W_dt, b_dt, A_log, D_param, W_out):
    f32 = np.float32
    inputs = np.asarray(inputs, f32)
    W_in = np.asarray(W_in, f32)
    ck = np.asarray(conv_k, f32).reshape(D_CONV, D_INT)
    cb = np.asarray(conv_b, f32)
    W_x = np.asarray(W_x, f32)
    W_dt = np.asarray(W_dt, f32)
    b_dt = np.asarray(b_dt, f32)
    A = -np.exp(np.asarray(A_log, f32))
    D_param = np.asarray(D_param, f32)
    W_out = np.asarray(W_out, f32)

    in_maps = []
    for core in range(N_CORES):
        b, dh = divmod(core, 2)
        perm = np.concatenate([np.arange(dh * DH, (dh + 1) * DH),
                               np.arange((1 - dh) * DH, (2 - dh) * DH)])
        half = perm[:DH]
        w_in_p = np.concatenate([W_in[:, :D_INT][:, perm], W_in[:, D_INT:][:, perm]],
                                axis=1)
        ckp = ck[:, perm]                      # [4, 512]
        ckd = np.zeros((4, D_CONV, P, P), f32)
        for t in range(4):
            for w in range(D_CONV):
                np.fill_diagonal(ckd[t, w], ckp[w, t * P:(t + 1) * P])
        w_out4 = np.stack([
            W_out[half[0:P]], W_out[half[P:2 * P]],
            W_out[D_INT + half[0:P]], W_out[D_INT + half[P:2 * P]],
        ])
        dd = np.zeros((NDT, P, P), f32)
        for t in range(NDT):
            np.fill_diagonal(dd[t], D_param[half[t * P:(t + 1) * P]])
        bf16 = ml_dtypes.bfloat16
        in_maps.append({
            "ddiag": dd.astype(bf16),
            "inpT": np.ascontiguousarray(inputs[b].T).astype(bf16),
            "w_in": np.ascontiguousarray(w_in_p).astype(bf16),
            "ckd": ckd.astype(bf16),
            "cb": np.ascontiguousarray(cb[perm][:, None]),
            "w_x": np.ascontiguousarray(W_x[perm]).astype(bf16),
            "w_dt": np.ascontiguousarray(W_dt[:, half]).astype(bf16),
            "bdt": np.ascontiguousarray(b_dt[half][:, None]),
            "a": np.ascontiguousarray(A[half]),
            "w_out": np.ascontiguousarray(w_out4).astype(bf16),
        })
    return in_maps


LAST_EXEC_NS = None


def kernel(**inputs):
    global LAST_EXEC_NS
    import os
    from concourse.bass_utils import run_bass_kernel_spmd

    if "nc" not in _cache:
        _cache["nc"] = _build_program()
    nc = _cache["nc"]
    in_maps = _shard_inputs(**inputs)
    trace = bool(int(os.environ.get("BIMAMBA_TRACE", "0")))
    res = run_bass_kernel_spmd(nc, in_maps, core_ids=list(range(N_CORES)), trace=trace)
    _cache["last_res"] = res
    LAST_EXEC_NS = res.exec_time_ns
    out = np.zeros((B_SZ, L, D_IN), np.float32)
    for b in range(B_SZ):
        out[b] = res.results[2 * b]["out_part"] + res.results[2 * b + 1]["out_part"]
    return out
